# revision 8
# baseline (speedup 1.0000x reference)
"""BioRNN Trainium2 kernel: 8-core tensor-parallel recurrent scan.

Strategy
--------
N = 3840 neurons, padded to 4096 = 8 cores x 512 columns. Columns are
permuted so that each core owns matched slices of every region
(sr_esoma / sr_edend b0 / b1 / sr_inh / pfc_esoma / pfc_edend b0 / b1 /
pfc_inh) -> the dendrite->soma coupling is core-local.

w_eff = |w_rec| * mask is computed on host, column-sharded and kept
SBUF-resident ([4096, 512] f32 per core).  Each timestep:
  pre[64, 512] = x_t @ w_in_c + h_{t-1} @ W_c      (PSUM accumulation,
      lhsT = transposed h k-tiles from the previous all-gather)
  elementwise biology (i_me update, tanh dendrites, relu somas) -> r
  h_t = 0.8 h + 0.2 r  (natural layout, core-local)
  PE-transpose h_t chunk -> [512, 64] -> DRAM bounce -> AllGather(8)
  -> DMA gathered h_t^T back to SBUF for the next step's matmuls
  readout y_t = w_out^T @ h_t[sr_esoma]  (8 tiny matmuls, redundant
      on every core; accumulated in SBUF, one DMA at the end)
"""

import os
import sys

import numpy as np

sys.path.insert(0, "/opt/trn_rl_repo")

# ---------------- fixed architecture constants (from the model) ----------
N_IN = 128
N_OUT = 4
N = 3840
NP = 4096            # padded
B = 64
NCORES = 8
CPC = 512            # padded columns per core
DECAY = np.float32(10.0 / 50.0)      # 0.2
NET_NOISE = np.float32(0.01)

# original column-space regions
SR_ESOMA = (0, 512)
SR_EDEND = (512, 1536)
SR_INH = (1536, 1920)
PFC_ESOMA = (1920, 2432)
PFC_EDEND = (2432, 3456)
PFC_INH = (3456, 3840)

# per-core local layout (within the 512 owned columns):
# [0:64]    sr_edend_b0     [64:128]  sr_edend_b1
# [128:192] pfc_edend_b0    [192:256] pfc_edend_b1
# [256:320] sr_esoma        [320:384] pfc_esoma
# [384:432] sr_inh          [432:480] pfc_inh
# [480:512] dead (zero forever)

USE_F32R = True      # reduced-precision (tf32-like) matmul mode for speed


def _core_orig_cols(c):
    """Original column indices owned by core c, in local layout order."""
    i64 = np.arange(64)
    i48 = np.arange(48)
    return np.concatenate([
        512 + c * 64 + i64,          # sr_edend b0
        1024 + c * 64 + i64,         # sr_edend b1
        2432 + c * 64 + i64,         # pfc_edend b0
        2944 + c * 64 + i64,         # pfc_edend b1
        0 + c * 64 + i64,            # sr_esoma
        1920 + c * 64 + i64,         # pfc_esoma
        1536 + c * 48 + i48,         # sr_inh
        3456 + c * 48 + i48,         # pfc_inh
    ])


def _perm_tables():
    """gather[newp] = orig index or -1 (dead); pos[orig-permuted-order]."""
    gather = np.full(NP, -1, dtype=np.int64)
    for c in range(NCORES):
        cols = _core_orig_cols(c)
        gather[c * CPC: c * CPC + 480] = cols
    valid = gather >= 0
    return gather, valid


_GATHER, _VALID = _perm_tables()


# ---------------- bass program (built lazily, cached per T) --------------
_PROGRAM_CACHE = {}
LAST_EXEC_NS = None
LAST_RESULTS = None


def _build_program(T):
    import concourse.bacc as bacc
    import concourse.bass as bass
    import concourse.tile as tile
    from concourse import mybir

    f32 = mybir.dt.float32
    f32r = mybir.dt.float32r
    AF = mybir.ActivationFunctionType
    ALU = mybir.AluOpType

    nc = bacc.Bacc(
        "TRN2",
        target_bir_lowering=False,
        debug=False,
        enable_asserts=False,
        num_devices=NCORES,
    )

    # ---- I/O -----------------------------------------------------------
    w_sh_d = nc.dram_tensor("w_shard", [32, 128, CPC], f32r, kind="ExternalInput")
    w_in_d = nc.dram_tensor("w_in_c", [128, CPC], f32r, kind="ExternalInput")
    noise_d = nc.dram_tensor("noise_c", [T, B, CPC], f32, kind="ExternalInput")
    xT_d = nc.dram_tensor("x_T", [T, N_IN, B], f32r, kind="ExternalInput")
    h0T_d = nc.dram_tensor("h0_T", [32, 128, B], f32r, kind="ExternalInput")
    h0n_d = nc.dram_tensor("h0n_c", [B, CPC], f32, kind="ExternalInput")
    ime0_d = nc.dram_tensor("ime0_c", [B, 128], f32, kind="ExternalInput")
    alpha_d = nc.dram_tensor("alpha_c", [B, 128], f32, kind="ExternalInput")
    d2s_d = nc.dram_tensor("d2s", [B, 1], f32, kind="ExternalInput")
    wout_d = nc.dram_tensor("w_out_pk", [64, 32], f32r, kind="ExternalInput")
    ident_d = nc.dram_tensor("ident", [64, 64], f32, kind="ExternalInput")
    y_d = nc.dram_tensor("y_out", [4, T * B], f32, kind="ExternalOutput")

    rg = [list(range(NCORES))]


    with tile.TileContext(nc) as tc:
        with (
            tc.tile_pool(name="const", bufs=1) as constp,
            tc.tile_pool(name="state", bufs=1) as statep,
            tc.tile_pool(name="hTp", bufs=2) as hTp,
            tc.tile_pool(name="iop", bufs=3) as iop,
            tc.tile_pool(name="ewp", bufs=2) as ewp,
            tc.tile_pool(name="ps_pre", bufs=2, space="PSUM") as pspre,
            tc.tile_pool(name="ps_t", bufs=2, space="PSUM") as pst,
            tc.tile_pool(name="ps_y", bufs=2, space="PSUM") as psy,
            tc.tile_pool(name="dramp", bufs=2, space="DRAM") as dramp,
        ):
            # ---- constants / state preload -----------------------------
            w_sb = constp.tile([128, 32 * CPC], f32r, name="w_sb")
            for k in range(32):
                nc.sync.dma_start(
                    out=w_sb[:, k * CPC:(k + 1) * CPC], in_=w_sh_d[k]
                )
            w_in_sb = constp.tile([128, CPC], f32r, name="w_in_sb")
            nc.sync.dma_start(out=w_in_sb[:], in_=w_in_d[:])
            alpha_sb = constp.tile([B, 128], f32, name="alpha_sb")
            nc.sync.dma_start(out=alpha_sb[:], in_=alpha_d[:])
            d2s_sb = constp.tile([B, 1], f32, name="d2s_sb")
            nc.sync.dma_start(out=d2s_sb[:], in_=d2s_d[:])
            wout_sb = constp.tile([64, 32], f32r, name="wout_sb")
            nc.sync.dma_start(out=wout_sb[:], in_=wout_d[:])
            ident_sb = constp.tile([64, 64], f32, name="ident_sb")
            nc.sync.dma_start(out=ident_sb[:], in_=ident_d[:])

            h_sb = statep.tile([B, CPC], f32, name="h_sb")
            nc.sync.dma_start(out=h_sb[:], in_=h0n_d[:])
            ime_sb = statep.tile([B, 128], f32, name="ime_sb")
            nc.sync.dma_start(out=ime_sb[:], in_=ime0_d[:])
            y_sb = statep.tile([4, T * B], f32, name="y_sb")

            hT = hTp.tile([128, 32 * B], f32r, tag="hT", name="hT0")
            for k in range(32):
                nc.sync.dma_start(out=hT[:, k * B:(k + 1) * B], in_=h0T_d[k])

            for t in range(T):
                noise_sb = iop.tile([B, CPC], f32, tag="noise", name=f"nz{t}")
                nc.sync.dma_start(out=noise_sb[:], in_=noise_d[t])
                xT_sb = iop.tile([N_IN, B], f32r, tag="xT", name=f"xT{t}")
                nc.sync.dma_start(out=xT_sb[:], in_=xT_d[t])

                # ---- pre = x_t @ w_in + h @ W  (PSUM accumulation) -----
                pre_ps = pspre.tile([B, CPC], f32, tag="pre", name=f"pre{t}")
                nc.tensor.matmul(
                    pre_ps[:], xT_sb[:], w_in_sb[:],
                    start=True, stop=False,
                )
                for k in range(32):
                    nc.tensor.matmul(
                        pre_ps[:],
                        hT[:, k * B:(k + 1) * B],
                        w_sb[:, k * CPC:(k + 1) * CPC],
                        start=False, stop=(k == 31),
                    )

                # ---- elementwise biology -------------------------------
                pre_sb = ewp.tile([B, CPC], f32, tag="pre_sb", name=f"psb{t}")
                nc.vector.tensor_add(pre_sb[:], pre_ps[:], noise_sb[:])

                # i_me update (pfc_edend cols 128:256)
                tmp_d = ewp.tile([B, 128], f32, tag="tmp_d", name=f"td{t}")
                nc.vector.tensor_sub(tmp_d[:], pre_sb[:, 128:256], ime_sb[:])
                nc.vector.tensor_mul(tmp_d[:], tmp_d[:], alpha_sb[:])
                nc.vector.tensor_add(ime_sb[:], ime_sb[:], tmp_d[:])

                dpfc = ewp.tile([B, 128], f32, tag="dpfc", name=f"dp{t}")
                nc.vector.tensor_add(dpfc[:], pre_sb[:, 128:256], ime_sb[:])

                r_sb = ewp.tile([B, CPC], f32, tag="r", name=f"r{t}")
                nc.scalar.activation(r_sb[:, 0:128], pre_sb[:, 0:128], AF.Tanh)
                nc.scalar.activation(r_sb[:, 128:256], dpfc[:], AF.Tanh)

                dsum = ewp.tile([B, 128], f32, tag="dsum", name=f"ds{t}")
                nc.vector.tensor_add(dsum[:, 0:64], r_sb[:, 0:64], r_sb[:, 64:128])
                nc.vector.tensor_add(dsum[:, 64:128], r_sb[:, 128:192], r_sb[:, 192:256])
                soma_in = ewp.tile([B, 128], f32, tag="soma_in", name=f"si{t}")
                nc.vector.scalar_tensor_tensor(
                    soma_in[:], dsum[:], d2s_sb[:], pre_sb[:, 256:384],
                    ALU.mult, ALU.add,
                )
                # relus on DVE (keep ACT's table pinned to Tanh)
                nc.vector.tensor_scalar_max(r_sb[:, 256:384], soma_in[:], 0.0)
                nc.vector.tensor_scalar_max(r_sb[:, 384:512], pre_sb[:, 384:512], 0.0)

                # h = 0.2 * (4*h + r)  == 0.8h + 0.2r
                h4 = ewp.tile([B, CPC], f32, tag="h4", name=f"h4_{t}")
                nc.vector.scalar_tensor_tensor(
                    h4[:], h_sb[:], 4.0, r_sb[:], ALU.mult, ALU.add
                )
                nc.vector.tensor_scalar_mul(h_sb[:], h4[:], 0.2)

                # ---- transpose own chunk -> [512, 64] ------------------
                t_ps = pst.tile([128, 4 * B], f32, tag="tps", name=f"tp{t}")
                for j in range(4):
                    nc.tensor.transpose(
                        t_ps[:, j * B:(j + 1) * B],
                        h_sb[:, j * 128:(j + 1) * 128],
                        ident_sb[:],
                    )

                hTown = ewp.tile([128, 4 * B], f32r, tag="hTown", name=f"hto{t}")
                nc.vector.tensor_copy(hTown[:], t_ps[:])
                ag_in = dramp.tile([4, 128, B], f32r, tag="ag_in", name=f"agi{t}")
                for j in range(4):
                    nc.sync.dma_start(
                        out=ag_in[j], in_=hTown[:, j * B:(j + 1) * B]
                    )
                ag_out = dramp.tile(
                    [32, 128, B], f32r, tag="ag_out", name=f"ago{t}",
                    addr_space="Shared",
                )
                nc.gpsimd.collective_compute(
                    "AllGather",
                    ALU.bypass,
                    replica_groups=rg,
                    ins=[ag_in[:].opt()],
                    outs=[ag_out[:].opt()],
                )

                hT = hTp.tile([128, 32 * B], f32r, tag="hT", name=f"hTg{t}")
                for k in range(32):
                    nc.sync.dma_start(
                        out=hT[:, k * B:(k + 1) * B],
                        in_=ag_out[k],
                    )

                # ---- readout y_t = w_out^T @ h_t[sr_esoma] -------------
                y_ps = psy.tile([4, B], f32, tag="yps", name=f"yp{t}")
                for c in range(NCORES):
                    ktile = 4 * c + 2          # sr_esoma lives here
                    nc.tensor.matmul(
                        y_ps[:],
                        wout_sb[:, 4 * c:4 * (c + 1)],
                        hT[:64, ktile * B:ktile * B + B],
                        start=(c == 0), stop=(c == NCORES - 1),
                    )
                nc.vector.tensor_copy(y_sb[:, t * B:(t + 1) * B], y_ps[:])

            nc.sync.dma_start(out=y_d[:], in_=y_sb[:])

    nc.compile()
    return nc


def _get_program(T):
    if T not in _PROGRAM_CACHE:
        _PROGRAM_CACHE[T] = _build_program(T)
    return _PROGRAM_CACHE[T]


# ---------------- host-side prep ----------------------------------------
def _round_f32r(a):
    """Round fp32 to the PE's FP32R format: 8-bit exp, 11-bit mantissa
    (round-to-nearest, low 12 mantissa bits zeroed)."""
    a = np.ascontiguousarray(a, np.float32)
    u = a.view(np.uint32)
    shift = 12
    bias = ((u >> shift) & 1).astype(np.uint32) + np.uint32((1 << (shift - 1)) - 1)
    u2 = (u + bias) & np.uint32(0xFFFFF000)
    return u2.view(np.float32)


def _prep_inputs(x, h0, i_me0, noise, w_rec, w_in, b, w_out, mask,
                 alpha_me, dend2soma):
    T = x.shape[1]
    f32 = np.float32
    x = np.asarray(x, f32)
    h0 = np.asarray(h0, f32)
    i_me0 = np.asarray(i_me0, f32)
    noise = np.asarray(noise, f32)
    w_rec = np.asarray(w_rec, f32)
    w_in = np.asarray(w_in, f32)
    b = np.asarray(b, f32)
    w_out = np.asarray(w_out, f32)
    mask = np.asarray(mask, f32)
    alpha_me = np.asarray(alpha_me, f32)
    dend2soma = np.asarray(dend2soma, f32)

    w_eff = np.abs(w_rec) * mask                     # [N, N]

    ordr = _GATHER[_VALID]                           # permuted orig order
    pos = np.nonzero(_VALID)[0]

    w_pad = np.zeros((NP, NP), dtype=f32)
    w_pad[np.ix_(pos, pos)] = w_eff[np.ix_(ordr, ordr)]

    # replicated inputs
    xT = _round_f32r(x.transpose(1, 2, 0))                   # [T, 128, B]
    h0_pad = np.zeros((B, NP), dtype=f32)
    h0_pad[:, pos] = h0[:, ordr]
    h0T = _round_f32r(h0_pad.T).reshape(32, 128, B)
    ident = np.eye(64, dtype=f32)
    d2s = np.broadcast_to(dend2soma.reshape(1, 1), (B, 1)).copy()
    w_out_pk = np.zeros((64, 32), dtype=f32)
    for c in range(NCORES):
        w_out_pk[:, 4 * c:4 * (c + 1)] = w_out[c * 64:(c + 1) * 64, :]
    w_out_pk = _round_f32r(w_out_pk)

    in_maps = []
    for c in range(NCORES):
        cols = _core_orig_cols(c)                    # 480 orig col ids
        w_shard = _round_f32r(
            w_pad[:, c * CPC:(c + 1) * CPC]
        ).reshape(32, 128, CPC)

        noise_c = np.zeros((T, B, CPC), dtype=f32)
        noise_c[:, :, :480] = NET_NOISE * noise[:, :, cols] + b[cols]

        w_in_c = np.zeros((128, CPC), dtype=f32)
        w_in_c[:, :480] = w_in[:, cols]

        h0n_c = h0_pad[:, c * CPC:(c + 1) * CPC].copy()

        # i_me / alpha: pfc_edend slices (b0, b1) of this core
        sl0 = slice(c * 64, (c + 1) * 64)
        sl1 = slice(512 + c * 64, 512 + (c + 1) * 64)
        ime_c = np.concatenate([i_me0[:, sl0], i_me0[:, sl1]], axis=1)
        alpha_c = np.concatenate([alpha_me[sl0], alpha_me[sl1]])
        alpha_c = np.broadcast_to(alpha_c, (B, 128)).copy()

        in_maps.append({
            "w_shard": w_shard,
            "w_in_c": _round_f32r(w_in_c),
            "noise_c": noise_c,
            "x_T": xT,
            "h0_T": h0T,
            "h0n_c": np.ascontiguousarray(h0n_c),
            "ime0_c": np.ascontiguousarray(ime_c),
            "alpha_c": alpha_c,
            "d2s": d2s,
            "w_out_pk": w_out_pk,
            "ident": ident,
        })
    return in_maps, T


def _install_ntff_hook():
    """The agent image's antenv lacks axon_hooks; recreate it and wire the
    ctypes NTFF profiler from trn_boot (trace-only path)."""
    import types

    if "antenv.axon_hooks" in sys.modules:
        return
    import antenv

    mod = types.ModuleType("antenv.axon_hooks")
    _h = {"hook": None}
    mod.set_axon_ntff_profile_hook = lambda h: _h.__setitem__("hook", h)
    mod.get_axon_ntff_profile_hook = lambda: _h["hook"]
    sys.modules["antenv.axon_hooks"] = mod
    antenv.axon_hooks = mod
    try:
        from trn_agent_boot.trn_boot import _ntff_profile_via_ctypes

        hook = _ntff_profile_via_ctypes("/opt/axon/libaxon_pjrt.so")
        mod.set_axon_ntff_profile_hook(hook)
    except Exception as e:  # pragma: no cover
        print("ntff hook install failed:", e, file=sys.stderr)


def kernel(**inputs):
    global LAST_EXEC_NS, LAST_RESULTS
    from concourse import bass_utils
    from concourse.bass_utils import run_bass_kernel_spmd

    in_maps, T = _prep_inputs(**inputs)
    nc = _get_program(T)

    trace = bool(int(os.environ.get("BIORNN_TRACE", "0")))
    if trace:
        _install_ntff_hook()
        bass_utils.upload_artifacts = lambda d: d
    res = run_bass_kernel_spmd(
        nc, in_maps, core_ids=list(range(NCORES)), trace=trace
    )
    LAST_EXEC_NS = res.exec_time_ns
    LAST_RESULTS = res

    yT = res.results[0]["y_out"]                     # [4, T*B]
    y = yT.reshape(4, T, B).transpose(2, 1, 0)       # [B, T, 4]
    return np.ascontiguousarray(y.astype(np.float32))


# revision 11
# speedup vs baseline: 1.3324x; 1.3324x over previous
"""BioRNN Trainium2 kernel: 8-core tensor-parallel recurrent scan.

Strategy
--------
N = 3840 neurons, padded to 4096 = 8 cores x 512 columns. Columns are
permuted so that each core owns matched slices of every region
(sr_esoma / sr_edend b0 / b1 / sr_inh / pfc_esoma / pfc_edend b0 / b1 /
pfc_inh) -> the dendrite->soma coupling is core-local.

w_eff = |w_rec| * mask is computed on host, column-sharded and kept
SBUF-resident ([4096, 512] f32 per core).  Each timestep:
  pre[64, 512] = x_t @ w_in_c + h_{t-1} @ W_c      (PSUM accumulation,
      lhsT = transposed h k-tiles from the previous all-gather)
  elementwise biology (i_me update, tanh dendrites, relu somas) -> r
  h_t = 0.8 h + 0.2 r  (natural layout, core-local)
  PE-transpose h_t chunk -> [512, 64] -> DRAM bounce -> AllGather(8)
  -> DMA gathered h_t^T back to SBUF for the next step's matmuls
  readout y_t = w_out^T @ h_t[sr_esoma]  (8 tiny matmuls, redundant
      on every core; accumulated in SBUF, one DMA at the end)
"""

import os
import sys

import numpy as np

sys.path.insert(0, "/opt/trn_rl_repo")

# ---------------- fixed architecture constants (from the model) ----------
N_IN = 128
N_OUT = 4
N = 3840
NP = 4096            # padded
B = 64
NCORES = 8
CPC = 512            # padded columns per core
DECAY = np.float32(10.0 / 50.0)      # 0.2
NET_NOISE = np.float32(0.01)

# original column-space regions
SR_ESOMA = (0, 512)
SR_EDEND = (512, 1536)
SR_INH = (1536, 1920)
PFC_ESOMA = (1920, 2432)
PFC_EDEND = (2432, 3456)
PFC_INH = (3456, 3840)

# per-core local layout (within the 512 owned columns):
# [0:64]    sr_edend_b0     [64:128]  sr_edend_b1
# [128:192] pfc_edend_b0    [192:256] pfc_edend_b1
# [256:320] sr_esoma        [320:384] pfc_esoma
# [384:432] sr_inh          [432:480] pfc_inh
# [480:512] dead (zero forever)

USE_F32R = True      # reduced-precision (tf32-like) matmul mode for speed


def _core_orig_cols(c):
    """Original column indices owned by core c, in local layout order."""
    i64 = np.arange(64)
    i48 = np.arange(48)
    return np.concatenate([
        512 + c * 64 + i64,          # sr_edend b0
        1024 + c * 64 + i64,         # sr_edend b1
        2432 + c * 64 + i64,         # pfc_edend b0
        2944 + c * 64 + i64,         # pfc_edend b1
        0 + c * 64 + i64,            # sr_esoma
        1920 + c * 64 + i64,         # pfc_esoma
        1536 + c * 48 + i48,         # sr_inh
        3456 + c * 48 + i48,         # pfc_inh
    ])


def _perm_tables():
    """gather[newp] = orig index or -1 (dead); pos[orig-permuted-order]."""
    gather = np.full(NP, -1, dtype=np.int64)
    for c in range(NCORES):
        cols = _core_orig_cols(c)
        gather[c * CPC: c * CPC + 480] = cols
    valid = gather >= 0
    return gather, valid


_GATHER, _VALID = _perm_tables()


# ---------------- bass program (built lazily, cached per T) --------------
_PROGRAM_CACHE = {}
LAST_EXEC_NS = None
LAST_RESULTS = None


def _build_program(T):
    import concourse.bacc as bacc
    import concourse.bass as bass
    import concourse.tile as tile
    from concourse import mybir

    f32 = mybir.dt.float32
    f32r = mybir.dt.float32r
    AF = mybir.ActivationFunctionType
    ALU = mybir.AluOpType

    nc = bacc.Bacc(
        "TRN2",
        target_bir_lowering=False,
        debug=False,
        enable_asserts=False,
        num_devices=NCORES,
    )

    # ---- I/O -----------------------------------------------------------
    w_sh_d = nc.dram_tensor("w_shard", [32, 128, CPC], f32r, kind="ExternalInput")
    w_in_d = nc.dram_tensor("w_in_c", [128, CPC], f32r, kind="ExternalInput")
    noise_d = nc.dram_tensor("noise_c", [T, B, CPC], f32, kind="ExternalInput")
    xT_d = nc.dram_tensor("x_T", [T, N_IN, B], f32r, kind="ExternalInput")
    h0T_d = nc.dram_tensor("h0_T", [32, 128, B], f32r, kind="ExternalInput")
    h0n_d = nc.dram_tensor("h0n_c", [B, CPC], f32, kind="ExternalInput")
    ime0_d = nc.dram_tensor("ime0_c", [B, 128], f32, kind="ExternalInput")
    alpha_d = nc.dram_tensor("alpha_c", [B, 128], f32, kind="ExternalInput")
    d2s_d = nc.dram_tensor("d2s", [B, 1], f32, kind="ExternalInput")
    wout_d = nc.dram_tensor("w_out_pk", [64, 32], f32r, kind="ExternalInput")
    ident_d = nc.dram_tensor("ident", [64, 64], f32, kind="ExternalInput")
    y_d = nc.dram_tensor("y_out", [4, T * B], f32, kind="ExternalOutput")

    rg = [list(range(NCORES))]


    with tile.TileContext(nc) as tc:
        with (
            tc.tile_pool(name="const", bufs=1) as constp,
            tc.tile_pool(name="state", bufs=1) as statep,
            tc.tile_pool(name="hTp", bufs=2) as hTp,
            tc.tile_pool(name="iop", bufs=3) as iop,
            tc.tile_pool(name="ewp", bufs=2) as ewp,
            tc.tile_pool(name="ps_pre", bufs=2, space="PSUM") as pspre,
            tc.tile_pool(name="ps_t", bufs=2, space="PSUM") as pst,
            tc.tile_pool(name="ps_y", bufs=2, space="PSUM") as psy,
            tc.tile_pool(name="dramp", bufs=2, space="DRAM") as dramp,
        ):
            # ---- constants / state preload -----------------------------
            w_sb = constp.tile([128, 32 * CPC], f32r, name="w_sb")
            for k in range(32):
                nc.sync.dma_start(
                    out=w_sb[:, k * CPC:(k + 1) * CPC], in_=w_sh_d[k]
                )
            w_in_sb = constp.tile([128, CPC], f32r, name="w_in_sb")
            nc.sync.dma_start(out=w_in_sb[:], in_=w_in_d[:])
            alpha_sb = constp.tile([B, 128], f32, name="alpha_sb")
            nc.sync.dma_start(out=alpha_sb[:], in_=alpha_d[:])
            d2s_sb = constp.tile([B, 1], f32, name="d2s_sb")
            nc.sync.dma_start(out=d2s_sb[:], in_=d2s_d[:])
            wout_sb = constp.tile([64, 32], f32r, name="wout_sb")
            nc.sync.dma_start(out=wout_sb[:], in_=wout_d[:])
            ident_sb = constp.tile([64, 64], f32, name="ident_sb")
            nc.sync.dma_start(out=ident_sb[:], in_=ident_d[:])

            h_sb = statep.tile([B, CPC], f32, name="h_sb")
            nc.sync.dma_start(out=h_sb[:], in_=h0n_d[:])
            ime_sb = statep.tile([B, 128], f32, name="ime_sb")
            nc.sync.dma_start(out=ime_sb[:], in_=ime0_d[:])
            y_sb = statep.tile([4, T * B], f32, name="y_sb")

            hT = hTp.tile([128, 32 * B], f32r, tag="hT", name="hT0")
            for k in range(32):
                nc.sync.dma_start(out=hT[:, k * B:(k + 1) * B], in_=h0T_d[k])

            for t in range(T):
                if t == 0:
                    noise_sb = iop.tile([B, CPC], f32, tag="noise", name="nz0")
                    nc.gpsimd.dma_start(out=noise_sb[:], in_=noise_d[0])
                    xT_sb = iop.tile([N_IN, B], f32r, tag="xT", name="xT0")
                    nc.gpsimd.dma_start(out=xT_sb[:], in_=xT_d[0])
                    pre_ps = pspre.tile([B, CPC], f32, tag="pre", name="pre0")
                    nc.tensor.matmul(
                        pre_ps[:], xT_sb[:], w_in_sb[:], start=True, stop=False,
                    )
                else:
                    noise_sb, xT_sb, pre_ps = nxt_noise, nxt_xT, nxt_pre

                # ---- pre += h @ W  (PSUM accumulation over 32 k-tiles) --
                for k in range(32):
                    nc.tensor.matmul(
                        pre_ps[:],
                        hT[:, k * B:(k + 1) * B],
                        w_sb[:, k * CPC:(k + 1) * CPC],
                        start=False, stop=(k == 31),
                    )

                # ---- elementwise biology -------------------------------
                pre_sb = ewp.tile([B, CPC], f32, tag="pre_sb", name=f"psb{t}")
                nc.vector.tensor_add(pre_sb[:], pre_ps[:], noise_sb[:])

                # i_me update (pfc_edend cols 128:256)
                tmp_d = ewp.tile([B, 128], f32, tag="tmp_d", name=f"td{t}")
                nc.vector.tensor_sub(tmp_d[:], pre_sb[:, 128:256], ime_sb[:])
                nc.vector.tensor_mul(tmp_d[:], tmp_d[:], alpha_sb[:])
                nc.vector.tensor_add(ime_sb[:], ime_sb[:], tmp_d[:])

                dpfc = ewp.tile([B, 128], f32, tag="dpfc", name=f"dp{t}")
                nc.vector.tensor_add(dpfc[:], pre_sb[:, 128:256], ime_sb[:])

                r_sb = ewp.tile([B, CPC], f32, tag="r", name=f"r{t}")
                nc.scalar.activation(r_sb[:, 0:128], pre_sb[:, 0:128], AF.Tanh)
                nc.scalar.activation(r_sb[:, 128:256], dpfc[:], AF.Tanh)

                # h update first half (dendrite cols) -> transposes j=0,1
                h4a = ewp.tile([B, CPC], f32, tag="h4", name=f"h4_{t}")
                nc.vector.scalar_tensor_tensor(
                    h4a[:, 0:256], h_sb[:, 0:256], 4.0, r_sb[:, 0:256],
                    ALU.mult, ALU.add,
                )
                nc.vector.tensor_scalar_mul(h_sb[:, 0:256], h4a[:, 0:256], 0.2)

                t_ps = pst.tile([128, 4 * B], f32, tag="tps", name=f"tp{t}")
                for j in range(2):
                    nc.tensor.transpose(
                        t_ps[:, j * B:(j + 1) * B],
                        h_sb[:, j * 128:(j + 1) * 128],
                        ident_sb[:],
                    )

                dsum = ewp.tile([B, 128], f32, tag="dsum", name=f"ds{t}")
                nc.vector.tensor_add(dsum[:, 0:64], r_sb[:, 0:64], r_sb[:, 64:128])
                nc.vector.tensor_add(dsum[:, 64:128], r_sb[:, 128:192], r_sb[:, 192:256])
                nc.vector.scalar_tensor_tensor(
                    pre_sb[:, 256:384], dsum[:], d2s_sb[:], pre_sb[:, 256:384],
                    ALU.mult, ALU.add,
                )
                # relu on DVE (keep ACT's table pinned to Tanh)
                nc.vector.tensor_scalar_max(r_sb[:, 256:512], pre_sb[:, 256:512], 0.0)

                # h update second half -> transposes j=2,3
                nc.vector.scalar_tensor_tensor(
                    h4a[:, 256:512], h_sb[:, 256:512], 4.0, r_sb[:, 256:512],
                    ALU.mult, ALU.add,
                )
                nc.vector.tensor_scalar_mul(h_sb[:, 256:512], h4a[:, 256:512], 0.2)
                for j in range(2, 4):
                    nc.tensor.transpose(
                        t_ps[:, j * B:(j + 1) * B],
                        h_sb[:, j * 128:(j + 1) * 128],
                        ident_sb[:],
                    )

                # evacuate + round to f32r, ship to the all-gather
                hTown = ewp.tile([128, 4 * B], f32r, tag="hTown", name=f"hto{t}")
                nc.vector.tensor_copy(hTown[:], t_ps[:])
                ag_in = dramp.tile([128, 4 * B], f32r, tag="ag_in", name=f"agi{t}")
                for j, eng in enumerate((nc.sync, nc.scalar, nc.gpsimd, nc.sync)):
                    eng.dma_start(
                        out=ag_in[:, j * B:(j + 1) * B],
                        in_=hTown[:, j * B:(j + 1) * B],
                    )
                ag_out = dramp.tile(
                    [NCORES, 128, 4 * B], f32r, tag="ag_out", name=f"ago{t}",
                    addr_space="Shared",
                )
                nc.gpsimd.collective_compute(
                    "AllGather",
                    ALU.bypass,
                    replica_groups=rg,
                    ins=[ag_in[:].opt()],
                    outs=[ag_out[:].opt()],
                )

                # prefetch next step inputs while the collective flies
                if t + 1 < T:
                    nxt_noise = iop.tile([B, CPC], f32, tag="noise", name=f"nz{t+1}")
                    nc.gpsimd.dma_start(out=nxt_noise[:], in_=noise_d[t + 1])
                    nxt_xT = iop.tile([N_IN, B], f32r, tag="xT", name=f"xT{t+1}")
                    nc.gpsimd.dma_start(out=nxt_xT[:], in_=xT_d[t + 1])

                # readout for the PREVIOUS step fills the collective window
                if t > 0:
                    y_ps = psy.tile([4, B], f32, tag="yps", name=f"yp{t-1}")
                    for c in range(NCORES):
                        ktile = 4 * c + 2
                        nc.tensor.matmul(
                            y_ps[:],
                            wout_sb[:, 4 * c:4 * (c + 1)],
                            hT[:64, ktile * B:ktile * B + B],
                            start=(c == 0), stop=(c == NCORES - 1),
                        )
                    nc.vector.tensor_copy(y_sb[:, (t - 1) * B:t * B], y_ps[:])

                # next step's input-term matmul can also run during the AG
                if t + 1 < T:
                    nxt_pre = pspre.tile([B, CPC], f32, tag="pre", name=f"pre{t+1}")
                    nc.tensor.matmul(
                        nxt_pre[:], nxt_xT[:], w_in_sb[:], start=True, stop=False,
                    )

                # gather the new h^T (one natural [128, 256] DMA per rank)
                hT = hTp.tile([128, 32 * B], f32r, tag="hT", name=f"hTg{t}")
                engs = (nc.sync, nc.gpsimd, nc.scalar, nc.sync)
                for c in range(NCORES):
                    engs[c % 4].dma_start(
                        out=hT[:, c * 4 * B:(c + 1) * 4 * B],
                        in_=ag_out[c],
                    )

            # final readout (step T-1)
            y_ps = psy.tile([4, B], f32, tag="yps", name=f"yp{T-1}")
            for c in range(NCORES):
                ktile = 4 * c + 2
                nc.tensor.matmul(
                    y_ps[:],
                    wout_sb[:, 4 * c:4 * (c + 1)],
                    hT[:64, ktile * B:ktile * B + B],
                    start=(c == 0), stop=(c == NCORES - 1),
                )
            nc.vector.tensor_copy(y_sb[:, (T - 1) * B:T * B], y_ps[:])

            nc.sync.dma_start(out=y_d[:], in_=y_sb[:])

    nc.compile()
    return nc


def _get_program(T):
    if T not in _PROGRAM_CACHE:
        _PROGRAM_CACHE[T] = _build_program(T)
    return _PROGRAM_CACHE[T]


# ---------------- host-side prep ----------------------------------------
def _round_f32r(a):
    """Round fp32 to the PE's FP32R format: 8-bit exp, 11-bit mantissa
    (round-to-nearest, low 12 mantissa bits zeroed)."""
    a = np.ascontiguousarray(a, np.float32)
    u = a.view(np.uint32)
    shift = 12
    bias = ((u >> shift) & 1).astype(np.uint32) + np.uint32((1 << (shift - 1)) - 1)
    u2 = (u + bias) & np.uint32(0xFFFFF000)
    return u2.view(np.float32)


def _prep_inputs(x, h0, i_me0, noise, w_rec, w_in, b, w_out, mask,
                 alpha_me, dend2soma):
    T = x.shape[1]
    f32 = np.float32
    x = np.asarray(x, f32)
    h0 = np.asarray(h0, f32)
    i_me0 = np.asarray(i_me0, f32)
    noise = np.asarray(noise, f32)
    w_rec = np.asarray(w_rec, f32)
    w_in = np.asarray(w_in, f32)
    b = np.asarray(b, f32)
    w_out = np.asarray(w_out, f32)
    mask = np.asarray(mask, f32)
    alpha_me = np.asarray(alpha_me, f32)
    dend2soma = np.asarray(dend2soma, f32)

    w_eff = np.abs(w_rec) * mask                     # [N, N]

    ordr = _GATHER[_VALID]                           # permuted orig order
    pos = np.nonzero(_VALID)[0]

    w_pad = np.zeros((NP, NP), dtype=f32)
    w_pad[np.ix_(pos, pos)] = w_eff[np.ix_(ordr, ordr)]

    # replicated inputs
    xT = _round_f32r(x.transpose(1, 2, 0))                   # [T, 128, B]
    h0_pad = np.zeros((B, NP), dtype=f32)
    h0_pad[:, pos] = h0[:, ordr]
    h0T = _round_f32r(h0_pad.T).reshape(32, 128, B)
    ident = np.eye(64, dtype=f32)
    d2s = np.broadcast_to(dend2soma.reshape(1, 1), (B, 1)).copy()
    w_out_pk = np.zeros((64, 32), dtype=f32)
    for c in range(NCORES):
        w_out_pk[:, 4 * c:4 * (c + 1)] = w_out[c * 64:(c + 1) * 64, :]
    w_out_pk = _round_f32r(w_out_pk)

    in_maps = []
    for c in range(NCORES):
        cols = _core_orig_cols(c)                    # 480 orig col ids
        w_shard = _round_f32r(
            w_pad[:, c * CPC:(c + 1) * CPC]
        ).reshape(32, 128, CPC)

        noise_c = np.zeros((T, B, CPC), dtype=f32)
        noise_c[:, :, :480] = NET_NOISE * noise[:, :, cols] + b[cols]

        w_in_c = np.zeros((128, CPC), dtype=f32)
        w_in_c[:, :480] = w_in[:, cols]

        h0n_c = h0_pad[:, c * CPC:(c + 1) * CPC].copy()

        # i_me / alpha: pfc_edend slices (b0, b1) of this core
        sl0 = slice(c * 64, (c + 1) * 64)
        sl1 = slice(512 + c * 64, 512 + (c + 1) * 64)
        ime_c = np.concatenate([i_me0[:, sl0], i_me0[:, sl1]], axis=1)
        alpha_c = np.concatenate([alpha_me[sl0], alpha_me[sl1]])
        alpha_c = np.broadcast_to(alpha_c, (B, 128)).copy()

        in_maps.append({
            "w_shard": w_shard,
            "w_in_c": _round_f32r(w_in_c),
            "noise_c": noise_c,
            "x_T": xT,
            "h0_T": h0T,
            "h0n_c": np.ascontiguousarray(h0n_c),
            "ime0_c": np.ascontiguousarray(ime_c),
            "alpha_c": alpha_c,
            "d2s": d2s,
            "w_out_pk": w_out_pk,
            "ident": ident,
        })
    return in_maps, T


def _install_ntff_hook():
    """The agent image's antenv lacks axon_hooks; recreate it and wire the
    ctypes NTFF profiler from trn_boot (trace-only path)."""
    import types

    if "antenv.axon_hooks" in sys.modules:
        return
    import antenv

    mod = types.ModuleType("antenv.axon_hooks")
    _h = {"hook": None}
    mod.set_axon_ntff_profile_hook = lambda h: _h.__setitem__("hook", h)
    mod.get_axon_ntff_profile_hook = lambda: _h["hook"]
    sys.modules["antenv.axon_hooks"] = mod
    antenv.axon_hooks = mod
    try:
        from trn_agent_boot.trn_boot import _ntff_profile_via_ctypes

        hook = _ntff_profile_via_ctypes("/opt/axon/libaxon_pjrt.so")
        mod.set_axon_ntff_profile_hook(hook)
    except Exception as e:  # pragma: no cover
        print("ntff hook install failed:", e, file=sys.stderr)


def kernel(**inputs):
    global LAST_EXEC_NS, LAST_RESULTS
    from concourse import bass_utils
    from concourse.bass_utils import run_bass_kernel_spmd

    in_maps, T = _prep_inputs(**inputs)
    nc = _get_program(T)

    trace = bool(int(os.environ.get("BIORNN_TRACE", "0")))
    if trace:
        _install_ntff_hook()
        bass_utils.upload_artifacts = lambda d: d
    res = run_bass_kernel_spmd(
        nc, in_maps, core_ids=list(range(NCORES)), trace=trace
    )
    LAST_EXEC_NS = res.exec_time_ns
    LAST_RESULTS = res

    yT = res.results[0]["y_out"]                     # [4, T*B]
    y = yT.reshape(4, T, B).transpose(2, 1, 0)       # [B, T, 4]
    return np.ascontiguousarray(y.astype(np.float32))


# revision 13
# speedup vs baseline: 1.4831x; 1.1131x over previous
"""BioRNN Trainium2 kernel: 8-core tensor-parallel recurrent scan.

Strategy
--------
N = 3840 neurons, padded to 4096 = 8 cores x 512 columns. Columns are
permuted so that each core owns matched slices of every region
(sr_esoma / sr_edend b0 / b1 / sr_inh / pfc_esoma / pfc_edend b0 / b1 /
pfc_inh) -> the dendrite->soma coupling is core-local.

w_eff = |w_rec| * mask is computed on host, column-sharded and kept
SBUF-resident ([4096, 512] f32 per core).  Each timestep:
  pre[64, 512] = x_t @ w_in_c + h_{t-1} @ W_c      (PSUM accumulation,
      lhsT = transposed h k-tiles from the previous all-gather)
  elementwise biology (i_me update, tanh dendrites, relu somas) -> r
  h_t = 0.8 h + 0.2 r  (natural layout, core-local)
  PE-transpose h_t chunk -> [512, 64] -> DRAM bounce -> AllGather(8)
  -> DMA gathered h_t^T back to SBUF for the next step's matmuls
  readout y_t = w_out^T @ h_t[sr_esoma]  (8 tiny matmuls, redundant
      on every core; accumulated in SBUF, one DMA at the end)
"""

import os
import sys

import numpy as np

sys.path.insert(0, "/opt/trn_rl_repo")

# ---------------- fixed architecture constants (from the model) ----------
N_IN = 128
N_OUT = 4
N = 3840
NP = 4096            # padded
B = 64
NCORES = 8
CPC = 512            # padded columns per core
DECAY = np.float32(10.0 / 50.0)      # 0.2
NET_NOISE = np.float32(0.01)

# original column-space regions
SR_ESOMA = (0, 512)
SR_EDEND = (512, 1536)
SR_INH = (1536, 1920)
PFC_ESOMA = (1920, 2432)
PFC_EDEND = (2432, 3456)
PFC_INH = (3456, 3840)

# per-core local layout (within the 512 owned columns):
# [0:64]    sr_edend_b0     [64:128]  sr_edend_b1
# [128:192] pfc_edend_b0    [192:256] pfc_edend_b1
# [256:320] sr_esoma        [320:384] pfc_esoma
# [384:432] sr_inh          [432:480] pfc_inh
# [480:512] dead (zero forever)

USE_F32R = True      # reduced-precision (tf32-like) matmul mode for speed


def _core_orig_cols(c):
    """Original column indices owned by core c, in local layout order."""
    i64 = np.arange(64)
    i48 = np.arange(48)
    return np.concatenate([
        512 + c * 64 + i64,          # sr_edend b0
        1024 + c * 64 + i64,         # sr_edend b1
        2432 + c * 64 + i64,         # pfc_edend b0
        2944 + c * 64 + i64,         # pfc_edend b1
        0 + c * 64 + i64,            # sr_esoma
        1920 + c * 64 + i64,         # pfc_esoma
        1536 + c * 48 + i48,         # sr_inh
        3456 + c * 48 + i48,         # pfc_inh
    ])


def _perm_tables():
    """gather[newp] = orig index or -1 (dead); pos[orig-permuted-order]."""
    gather = np.full(NP, -1, dtype=np.int64)
    for c in range(NCORES):
        cols = _core_orig_cols(c)
        gather[c * CPC: c * CPC + 480] = cols
    valid = gather >= 0
    return gather, valid


_GATHER, _VALID = _perm_tables()


# ---------------- bass program (built lazily, cached per T) --------------
_PROGRAM_CACHE = {}
LAST_EXEC_NS = None
LAST_RESULTS = None


def _build_program(T):
    import concourse.bacc as bacc
    import concourse.bass as bass
    import concourse.tile as tile
    from concourse import mybir

    f32 = mybir.dt.float32
    f32r = mybir.dt.float32r
    AF = mybir.ActivationFunctionType
    ALU = mybir.AluOpType

    nc = bacc.Bacc(
        "TRN2",
        target_bir_lowering=False,
        debug=False,
        enable_asserts=False,
        num_devices=NCORES,
    )

    # ---- I/O -----------------------------------------------------------
    w_sh_d = nc.dram_tensor("w_shard", [32, 128, CPC], f32r, kind="ExternalInput")
    w_in_d = nc.dram_tensor("w_in_c", [128, CPC], f32r, kind="ExternalInput")
    noise_d = nc.dram_tensor("noise_c", [T, B, CPC], f32, kind="ExternalInput")
    xT_d = nc.dram_tensor("x_T", [T, N_IN, B], f32r, kind="ExternalInput")
    h0TA_d = nc.dram_tensor("h0_TA", [16, 128, B], f32r, kind="ExternalInput")
    h0TB_d = nc.dram_tensor("h0_TB", [16, 128, B], f32r, kind="ExternalInput")
    h0n_d = nc.dram_tensor("h0n_c", [B, CPC], f32, kind="ExternalInput")
    ime0_d = nc.dram_tensor("ime0_c", [B, 128], f32, kind="ExternalInput")
    alpha_d = nc.dram_tensor("alpha_c", [B, 128], f32, kind="ExternalInput")
    d2s_d = nc.dram_tensor("d2s", [B, 1], f32, kind="ExternalInput")
    wout_d = nc.dram_tensor("w_out_pk", [64, 32], f32r, kind="ExternalInput")
    ident_d = nc.dram_tensor("ident", [64, 64], f32, kind="ExternalInput")
    y_d = nc.dram_tensor("y_out", [4, T * B], f32, kind="ExternalOutput")

    rg = [list(range(NCORES))]


    with tile.TileContext(nc) as tc:
        with (
            tc.tile_pool(name="const", bufs=1) as constp,
            tc.tile_pool(name="state", bufs=1) as statep,
            tc.tile_pool(name="hTp", bufs=2) as hTp,
            tc.tile_pool(name="iop", bufs=3) as iop,
            tc.tile_pool(name="ewp", bufs=2) as ewp,
            tc.tile_pool(name="ps_pre", bufs=2, space="PSUM") as pspre,
            tc.tile_pool(name="ps_tA", bufs=2, space="PSUM") as pstA,
            tc.tile_pool(name="ps_tB", bufs=2, space="PSUM") as pstB,
            tc.tile_pool(name="ps_y", bufs=2, space="PSUM") as psy,
            tc.tile_pool(name="dramp", bufs=2, space="DRAM") as dramp,
        ):
            # ---- constants / state preload -----------------------------
            w_sb = constp.tile([128, 32 * CPC], f32r, name="w_sb")
            for k in range(32):
                nc.sync.dma_start(
                    out=w_sb[:, k * CPC:(k + 1) * CPC], in_=w_sh_d[k]
                )
            w_in_sb = constp.tile([128, CPC], f32r, name="w_in_sb")
            nc.sync.dma_start(out=w_in_sb[:], in_=w_in_d[:])
            alpha_sb = constp.tile([B, 128], f32, name="alpha_sb")
            nc.sync.dma_start(out=alpha_sb[:], in_=alpha_d[:])
            d2s_sb = constp.tile([B, 1], f32, name="d2s_sb")
            nc.sync.dma_start(out=d2s_sb[:], in_=d2s_d[:])
            wout_sb = constp.tile([64, 32], f32r, name="wout_sb")
            nc.sync.dma_start(out=wout_sb[:], in_=wout_d[:])
            ident_sb = constp.tile([64, 64], f32, name="ident_sb")
            nc.sync.dma_start(out=ident_sb[:], in_=ident_d[:])

            h_sb = statep.tile([B, CPC], f32, name="h_sb")
            nc.sync.dma_start(out=h_sb[:], in_=h0n_d[:])
            ime_sb = statep.tile([B, 128], f32, name="ime_sb")
            nc.sync.dma_start(out=ime_sb[:], in_=ime0_d[:])
            y_sb = statep.tile([4, T * B], f32, name="y_sb")

            hTA = hTp.tile([128, 16 * B], f32r, tag="hTA", name="hTA0")
            for k in range(16):
                nc.sync.dma_start(out=hTA[:, k * B:(k + 1) * B], in_=h0TA_d[k])
            hTB = hTp.tile([128, 16 * B], f32r, tag="hTB", name="hTB0")
            for k in range(16):
                nc.sync.dma_start(out=hTB[:, k * B:(k + 1) * B], in_=h0TB_d[k])

            for t in range(T):
                if t == 0:
                    noise_sb = iop.tile([B, CPC], f32, tag="noise", name="nz0")
                    nc.gpsimd.dma_start(out=noise_sb[:], in_=noise_d[0])
                    xT_sb = iop.tile([N_IN, B], f32r, tag="xT", name="xT0")
                    nc.gpsimd.dma_start(out=xT_sb[:], in_=xT_d[0])
                    pre_ps = pspre.tile([B, CPC], f32, tag="pre", name="pre0")
                    nc.tensor.matmul(
                        pre_ps[:], xT_sb[:], w_in_sb[:], start=True, stop=False,
                    )
                else:
                    noise_sb, xT_sb, pre_ps = nxt_noise, nxt_xT, nxt_pre

                # ---- pre += h @ W : A-phase tiles then B-phase tiles ----
                for c in range(NCORES):
                    for j in range(2):
                        nc.tensor.matmul(
                            pre_ps[:],
                            hTA[:, (2 * c + j) * B:(2 * c + j + 1) * B],
                            w_sb[:, (4 * c + j) * CPC:(4 * c + j + 1) * CPC],
                            start=False, stop=False,
                        )
                for c in range(NCORES):
                    for j in range(2):
                        nc.tensor.matmul(
                            pre_ps[:],
                            hTB[:, (2 * c + j) * B:(2 * c + j + 1) * B],
                            w_sb[:, (4 * c + 2 + j) * CPC:(4 * c + 3 + j) * CPC],
                            start=False, stop=(c == NCORES - 1 and j == 1),
                        )

                # ---- ew phase H1 (dendrite cols 0:256) ------------------
                pre_sb = ewp.tile([B, CPC], f32, tag="pre_sb", name=f"psb{t}")
                nc.vector.tensor_add(pre_sb[:, 0:256], pre_ps[:, 0:256], noise_sb[:, 0:256])

                tmp_d = ewp.tile([B, 128], f32, tag="tmp_d", name=f"td{t}")
                nc.vector.tensor_sub(tmp_d[:], pre_sb[:, 128:256], ime_sb[:])
                nc.vector.tensor_mul(tmp_d[:], tmp_d[:], alpha_sb[:])
                nc.vector.tensor_add(ime_sb[:], ime_sb[:], tmp_d[:])

                dpfc = ewp.tile([B, 128], f32, tag="dpfc", name=f"dp{t}")
                nc.vector.tensor_add(dpfc[:], pre_sb[:, 128:256], ime_sb[:])

                r_sb = ewp.tile([B, CPC], f32, tag="r", name=f"r{t}")
                nc.scalar.activation(r_sb[:, 0:128], pre_sb[:, 0:128], AF.Tanh)
                nc.scalar.activation(r_sb[:, 128:256], dpfc[:], AF.Tanh)

                h4a = ewp.tile([B, CPC], f32, tag="h4", name=f"h4_{t}")
                nc.vector.scalar_tensor_tensor(
                    h4a[:, 0:256], h_sb[:, 0:256], 4.0, r_sb[:, 0:256],
                    ALU.mult, ALU.add,
                )
                nc.vector.tensor_scalar_mul(h_sb[:, 0:256], h4a[:, 0:256], 0.2)

                t_psA = pstA.tile([128, 2 * B], f32, tag="tpsA", name=f"tpA{t}")
                for j in range(2):
                    nc.tensor.transpose(
                        t_psA[:, j * B:(j + 1) * B],
                        h_sb[:, j * 128:(j + 1) * 128],
                        ident_sb[:],
                    )
                hTownA = ewp.tile([128, 2 * B], f32r, tag="hTownA", name=f"htoA{t}")
                nc.vector.tensor_copy(hTownA[:], t_psA[:])
                ag_inA = dramp.tile([128, 2 * B], f32r, tag="ag_inA", name=f"agiA{t}")
                nc.sync.dma_start(out=ag_inA[:, 0:B], in_=hTownA[:, 0:B])
                nc.scalar.dma_start(out=ag_inA[:, B:2 * B], in_=hTownA[:, B:2 * B])
                ag_outA = dramp.tile(
                    [NCORES, 128, 2 * B], f32r, tag="ag_outA", name=f"agoA{t}",
                    addr_space="Shared",
                )
                nc.gpsimd.collective_compute(
                    "AllGather", ALU.bypass, replica_groups=rg,
                    ins=[ag_inA[:].opt()], outs=[ag_outA[:].opt()],
                )

                # ---- ew phase H2 (soma/inh cols 256:512) ----------------
                nc.vector.tensor_add(pre_sb[:, 256:512], pre_ps[:, 256:512], noise_sb[:, 256:512])
                dsum = ewp.tile([B, 128], f32, tag="dsum", name=f"ds{t}")
                nc.vector.tensor_add(dsum[:, 0:64], r_sb[:, 0:64], r_sb[:, 64:128])
                nc.vector.tensor_add(dsum[:, 64:128], r_sb[:, 128:192], r_sb[:, 192:256])
                nc.vector.scalar_tensor_tensor(
                    pre_sb[:, 256:384], dsum[:], d2s_sb[:], pre_sb[:, 256:384],
                    ALU.mult, ALU.add,
                )
                nc.vector.tensor_scalar_max(r_sb[:, 256:512], pre_sb[:, 256:512], 0.0)

                nc.vector.scalar_tensor_tensor(
                    h4a[:, 256:512], h_sb[:, 256:512], 4.0, r_sb[:, 256:512],
                    ALU.mult, ALU.add,
                )
                nc.vector.tensor_scalar_mul(h_sb[:, 256:512], h4a[:, 256:512], 0.2)
                t_psB = pstB.tile([128, 2 * B], f32, tag="tpsB", name=f"tpB{t}")
                for j in range(2):
                    nc.tensor.transpose(
                        t_psB[:, j * B:(j + 1) * B],
                        h_sb[:, (j + 2) * 128:(j + 3) * 128],
                        ident_sb[:],
                    )
                hTownB = ewp.tile([128, 2 * B], f32r, tag="hTownB", name=f"htoB{t}")
                nc.vector.tensor_copy(hTownB[:], t_psB[:])
                ag_inB = dramp.tile([128, 2 * B], f32r, tag="ag_inB", name=f"agiB{t}")
                nc.sync.dma_start(out=ag_inB[:, 0:B], in_=hTownB[:, 0:B])
                nc.scalar.dma_start(out=ag_inB[:, B:2 * B], in_=hTownB[:, B:2 * B])
                ag_outB = dramp.tile(
                    [NCORES, 128, 2 * B], f32r, tag="ag_outB", name=f"agoB{t}",
                    addr_space="Shared",
                )
                nc.gpsimd.collective_compute(
                    "AllGather", ALU.bypass, replica_groups=rg,
                    ins=[ag_inB[:].opt()], outs=[ag_outB[:].opt()],
                )

                # prefetch next step inputs while the collectives fly
                if t + 1 < T:
                    nxt_noise = iop.tile([B, CPC], f32, tag="noise", name=f"nz{t+1}")
                    nc.gpsimd.dma_start(out=nxt_noise[:], in_=noise_d[t + 1])
                    nxt_xT = iop.tile([N_IN, B], f32r, tag="xT", name=f"xT{t+1}")
                    nc.gpsimd.dma_start(out=nxt_xT[:], in_=xT_d[t + 1])

                # readout for the PREVIOUS step fills the collective window
                if t > 0:
                    y_ps = psy.tile([4, B], f32, tag="yps", name=f"yp{t-1}")
                    for c in range(NCORES):
                        nc.tensor.matmul(
                            y_ps[:],
                            wout_sb[:, 4 * c:4 * (c + 1)],
                            hTB[:64, (2 * c) * B:(2 * c) * B + B],
                            start=(c == 0), stop=(c == NCORES - 1),
                        )
                    nc.vector.tensor_copy(y_sb[:, (t - 1) * B:t * B], y_ps[:])

                # next step's input-term matmul can also run during the AG
                if t + 1 < T:
                    nxt_pre = pspre.tile([B, CPC], f32, tag="pre", name=f"pre{t+1}")
                    nc.tensor.matmul(
                        nxt_pre[:], nxt_xT[:], w_in_sb[:], start=True, stop=False,
                    )

                # gather phase A then phase B (natural [128, 128] per rank)
                hTA = hTp.tile([128, 16 * B], f32r, tag="hTA", name=f"hTAg{t}")
                engs = (nc.sync, nc.scalar)
                for c in range(NCORES):
                    engs[c % 2].dma_start(
                        out=hTA[:, c * 2 * B:(c + 1) * 2 * B],
                        in_=ag_outA[c],
                    )
                hTB = hTp.tile([128, 16 * B], f32r, tag="hTB", name=f"hTBg{t}")
                for c in range(NCORES):
                    engs[c % 2].dma_start(
                        out=hTB[:, c * 2 * B:(c + 1) * 2 * B],
                        in_=ag_outB[c],
                    )

            # final readout (step T-1)
            y_ps = psy.tile([4, B], f32, tag="yps", name=f"yp{T-1}")
            for c in range(NCORES):
                nc.tensor.matmul(
                    y_ps[:],
                    wout_sb[:, 4 * c:4 * (c + 1)],
                    hTB[:64, (2 * c) * B:(2 * c) * B + B],
                    start=(c == 0), stop=(c == NCORES - 1),
                )
            nc.vector.tensor_copy(y_sb[:, (T - 1) * B:T * B], y_ps[:])

            nc.sync.dma_start(out=y_d[:], in_=y_sb[:])

    nc.compile()
    return nc


def _get_program(T):
    if T not in _PROGRAM_CACHE:
        _PROGRAM_CACHE[T] = _build_program(T)
    return _PROGRAM_CACHE[T]


# ---------------- host-side prep ----------------------------------------
def _round_f32r(a):
    """Round fp32 to the PE's FP32R format: 8-bit exp, 11-bit mantissa
    (round-to-nearest, low 12 mantissa bits zeroed)."""
    a = np.ascontiguousarray(a, np.float32)
    u = a.view(np.uint32)
    shift = 12
    bias = ((u >> shift) & 1).astype(np.uint32) + np.uint32((1 << (shift - 1)) - 1)
    u2 = (u + bias) & np.uint32(0xFFFFF000)
    return u2.view(np.float32)


def _prep_inputs(x, h0, i_me0, noise, w_rec, w_in, b, w_out, mask,
                 alpha_me, dend2soma):
    T = x.shape[1]
    f32 = np.float32
    x = np.asarray(x, f32)
    h0 = np.asarray(h0, f32)
    i_me0 = np.asarray(i_me0, f32)
    noise = np.asarray(noise, f32)
    w_rec = np.asarray(w_rec, f32)
    w_in = np.asarray(w_in, f32)
    b = np.asarray(b, f32)
    w_out = np.asarray(w_out, f32)
    mask = np.asarray(mask, f32)
    alpha_me = np.asarray(alpha_me, f32)
    dend2soma = np.asarray(dend2soma, f32)

    w_eff = np.abs(w_rec) * mask                     # [N, N]

    ordr = _GATHER[_VALID]                           # permuted orig order
    pos = np.nonzero(_VALID)[0]

    w_pad = np.zeros((NP, NP), dtype=f32)
    w_pad[np.ix_(pos, pos)] = w_eff[np.ix_(ordr, ordr)]

    # replicated inputs
    xT = _round_f32r(x.transpose(1, 2, 0))                   # [T, 128, B]
    h0_pad = np.zeros((B, NP), dtype=f32)
    h0_pad[:, pos] = h0[:, ordr]
    h0T = _round_f32r(h0_pad.T).reshape(32, 128, B)
    idxA = [4 * c + j for c in range(NCORES) for j in (0, 1)]
    idxB = [4 * c + 2 + j for c in range(NCORES) for j in (0, 1)]
    h0TA = np.ascontiguousarray(h0T[idxA])
    h0TB = np.ascontiguousarray(h0T[idxB])
    ident = np.eye(64, dtype=f32)
    d2s = np.broadcast_to(dend2soma.reshape(1, 1), (B, 1)).copy()
    w_out_pk = np.zeros((64, 32), dtype=f32)
    for c in range(NCORES):
        w_out_pk[:, 4 * c:4 * (c + 1)] = w_out[c * 64:(c + 1) * 64, :]
    w_out_pk = _round_f32r(w_out_pk)

    in_maps = []
    for c in range(NCORES):
        cols = _core_orig_cols(c)                    # 480 orig col ids
        w_shard = _round_f32r(
            w_pad[:, c * CPC:(c + 1) * CPC]
        ).reshape(32, 128, CPC)

        noise_c = np.zeros((T, B, CPC), dtype=f32)
        noise_c[:, :, :480] = NET_NOISE * noise[:, :, cols] + b[cols]

        w_in_c = np.zeros((128, CPC), dtype=f32)
        w_in_c[:, :480] = w_in[:, cols]

        h0n_c = h0_pad[:, c * CPC:(c + 1) * CPC].copy()

        # i_me / alpha: pfc_edend slices (b0, b1) of this core
        sl0 = slice(c * 64, (c + 1) * 64)
        sl1 = slice(512 + c * 64, 512 + (c + 1) * 64)
        ime_c = np.concatenate([i_me0[:, sl0], i_me0[:, sl1]], axis=1)
        alpha_c = np.concatenate([alpha_me[sl0], alpha_me[sl1]])
        alpha_c = np.broadcast_to(alpha_c, (B, 128)).copy()

        in_maps.append({
            "w_shard": w_shard,
            "w_in_c": _round_f32r(w_in_c),
            "noise_c": noise_c,
            "x_T": xT,
            "h0_TA": h0TA,
            "h0_TB": h0TB,
            "h0n_c": np.ascontiguousarray(h0n_c),
            "ime0_c": np.ascontiguousarray(ime_c),
            "alpha_c": alpha_c,
            "d2s": d2s,
            "w_out_pk": w_out_pk,
            "ident": ident,
        })
    return in_maps, T


def _install_ntff_hook():
    """The agent image's antenv lacks axon_hooks; recreate it and wire the
    ctypes NTFF profiler from trn_boot (trace-only path)."""
    import types

    if "antenv.axon_hooks" in sys.modules:
        return
    import antenv

    mod = types.ModuleType("antenv.axon_hooks")
    _h = {"hook": None}
    mod.set_axon_ntff_profile_hook = lambda h: _h.__setitem__("hook", h)
    mod.get_axon_ntff_profile_hook = lambda: _h["hook"]
    sys.modules["antenv.axon_hooks"] = mod
    antenv.axon_hooks = mod
    try:
        from trn_agent_boot.trn_boot import _ntff_profile_via_ctypes

        hook = _ntff_profile_via_ctypes("/opt/axon/libaxon_pjrt.so")
        mod.set_axon_ntff_profile_hook(hook)
    except Exception as e:  # pragma: no cover
        print("ntff hook install failed:", e, file=sys.stderr)


def kernel(**inputs):
    global LAST_EXEC_NS, LAST_RESULTS
    from concourse import bass_utils
    from concourse.bass_utils import run_bass_kernel_spmd

    in_maps, T = _prep_inputs(**inputs)
    nc = _get_program(T)

    trace = bool(int(os.environ.get("BIORNN_TRACE", "0")))
    if trace:
        _install_ntff_hook()
        bass_utils.upload_artifacts = lambda d: d
    res = run_bass_kernel_spmd(
        nc, in_maps, core_ids=list(range(NCORES)), trace=trace
    )
    LAST_EXEC_NS = res.exec_time_ns
    LAST_RESULTS = res

    yT = res.results[0]["y_out"]                     # [4, T*B]
    y = yT.reshape(4, T, B).transpose(2, 1, 0)       # [B, T, 4]
    return np.ascontiguousarray(y.astype(np.float32))
